# revision 34
# baseline (speedup 1.0000x reference)
"""MultiHeadExternalAttention Trainium2 kernel (fp8 DoubleRow pipeline).

Math (exact algebraic refactor of the reference):
  h = x @ W_in + b_in feeds ONLY the mk projection, and the mv/out_proj pair
  is linear in attn.  Fold on the host (float64):
    logits = x @ (W_in_h @ W_mk) + (b_in_h @ W_mk + b_mk)    -> K=512, M=256
    y = attnL1_all[n,256] @ V[256,512] + b_y                 -> K=256, N=512
  where V = stack_h(W_mv @ W_out_h), b_y = b_out + tile(b_mv) @ W_out.

Precision/scale plan (end-to-end rel err ~8.9e-3 in numpy emulation, gate 2e-2):
  GEMM1: fp8e4m3 DoubleRow (x fp8, W_comb*64 fp8) -> psum = 64*logits;
         exp = Exp(psum/64 + bc) on ACT (scale folds the 64 back out),
         f32r out, accum_out gives the softmax denominator D pre-quantization.
  L1 norm: lcs = (mask/16)*rd f32r; s_psum = lcs^T exp (f32r matmul) = s/16;
         rs = 1/s_psum = 16/s (DVE reciprocal, bf16); po = rs broadcast to
         128 partitions - for steady batches via a stride-0 SBUF->SBUF DMA,
         for the fill/drain batches via a PE "outer" matmul (maskT^T @ rs)
         into spare psum banks (skips the ~2.2us DMA init latency where the
         pipeline cannot hide it); attnf = fp8(exp * rd * po) = 16*attn as a
         DVE scalar_tensor_tensor (GPSIMD cannot run TPB tensor ops or touch
         PSUM - verified against walrus codegen).
  GEMM2: fp8 DoubleRow folding both head-halves in one matmul:
         lhsT = attnf[:, :, 128j:128(j+1)], rhs = vv*64 fp8 -> y_psum = 1024*y.
  y: evicted psum->SBUF bf16 on ACT/DVE, DMA'd bf16; host does y/1024 + b_y.

Cost-model facts this design exploits (CoreSim V1 is the timing source here):
  - matmul cost = out_free_size * pe_cycle * cycles_per_row; K and M are free;
    fp8 DoubleRow = 0.5 cycles/row; f32r = 1.0 at N>=256.  GEMM1 drops 4x
    (k-pairs fold + double pump), GEMM2 drops 4x (t-fold + double pump).
  - DMA cost = free-dim bytes * 0.3855ns (partition dim uncharged), on the
    issuing engine's timeline; stride-0 src dims make broadcasts ~free.
  - engine op cost = free-size * cycle (+psum/sbuf access cycles on ACT/DVE);
    GPSIMD is cheapest (no access-cycle errata) but SBUF-only.

Sharding: pure data-parallel over batch, 4 batches per core, 8 cores.
"""

import numpy as np

B, N, E = 32, 1024, 512
H, HD, M = 16, 128, 16
NCORES = 8
BPC = B // NCORES  # batches per core

# ws column layout: bc [128, 2] then mask2 [128, 2, 16] (= mask/16)
_BC0 = 0
_MK0 = 2
_WS_COLS = 34

Y_SCALE = 1024.0  # attnf carries 16x, vv carries 64x


def round_f32r(a):
    """Round float32 array to float32r (11-bit mantissa, RNE)."""
    a = np.ascontiguousarray(a, dtype=np.float32)
    u = a.view(np.uint32)
    lsb = (u >> 12) & 1
    u2 = (u + 0x7FF + lsb) & np.uint32(0xFFFFF000)
    return u2.view(np.float32)


_nc_cache = {}


def _build_program(evict_acts=(0, 1, 2, 3, 4, 6, 7),
                   late_evict_acts=(0, 1, 2, 3, 4, 6, 7),
                   ydma_split=("gpsimd", "gpsimd", "sync", "gpsimd"),
                   bcast_eng=("sync", "gpsimd"), ps_banks=(2, 1, 3),
                   x0_split=False, evict_split=0,
                   drain_dma=("sync", "gpsimd", "scalar", "sync"),
                   drain_acts=(0, 2), all_outer=False, sbuf_bufs=(3, 3, 3),
                   rsp_bufs=2, ygroup=2):
    key = (evict_acts, late_evict_acts, ydma_split, bcast_eng, ps_banks,
           x0_split, evict_split, drain_dma, drain_acts, all_outer, sbuf_bufs,
           rsp_bufs, ygroup)
    if key in _nc_cache:
        return _nc_cache[key]
    import concourse.tile as tile
    from concourse import bacc, mybir

    f32 = mybir.dt.float32
    f32r = mybir.dt.float32r
    f8 = mybir.dt.float8e4
    bf16 = mybir.dt.bfloat16
    DR = mybir.MatmulPerfMode.DoubleRow
    Exp = mybir.ActivationFunctionType.Exp
    Copy = mybir.ActivationFunctionType.Copy
    mult = mybir.AluOpType.mult

    nc = bacc.Bacc("TRN2", target_bir_lowering=False, debug=False)

    xt = nc.dram_tensor("xt", [BPC, 512, 1024], f8, kind="ExternalInput").ap()
    wc = nc.dram_tensor("wc", [128, 2, 4, 128], f8, kind="ExternalInput").ap()
    vv = nc.dram_tensor("vv", [128, 2, 512], f8, kind="ExternalInput").ap()
    ws = nc.dram_tensor("ws", [128, _WS_COLS], f32, kind="ExternalInput").ap()
    mt = nc.dram_tensor("mt", [16, 2, 128], bf16, kind="ExternalInput").ap()
    y = nc.dram_tensor("y", [BPC, 1024, 512], bf16, kind="ExternalOutput").ap()

    NB = BPC

    with tile.TileContext(nc) as tc:
        with (
            tc.tile_pool(name="singles", bufs=1) as singles,
            tc.tile_pool(name="xtp", bufs=sbuf_bufs[0]) as xtp,
            tc.tile_pool(name="expp", bufs=sbuf_bufs[1]) as expp,
            tc.tile_pool(name="attnfp", bufs=sbuf_bufs[2]) as attnfp,
            tc.tile_pool(name="pop", bufs=8) as pop,
            tc.tile_pool(name="rsp", bufs=rsp_bufs) as rsp,
            tc.tile_pool(name="ygp", bufs=8) as ygp,
            tc.tile_pool(name="smallp", bufs=24) as smallp,
            tc.tile_pool(name="ps_pa", bufs=ps_banks[0], space="PSUM") as ps_pap,
            tc.tile_pool(name="ps_s", bufs=ps_banks[1], space="PSUM") as ps_sp,
            tc.tile_pool(name="ps_y", bufs=ps_banks[2], space="PSUM") as ps_yp,
            nc.allow_low_precision(reason="fp8 matmul operand chain"),
        ):
            # preload the exp table on ACT while the first DMAs stream
            dummy = smallp.tile([128, 1], f32, tag="dummy")
            nc.vector.memset(dummy, 0.0)
            dummy2 = smallp.tile([128, 1], f32, tag="dummy2")
            nc.scalar.activation(out=dummy2, in_=dummy, func=Exp, bias=0.0,
                                 scale=1.0)

            # weights + small constants first (GEMM1 gate), then x(0)
            wc_sb = singles.tile([128, 2, 4, 128], f8, tag="wc")
            ws_sb = singles.tile([128, _WS_COLS], f32, tag="ws")
            vv_sb = singles.tile([128, 2, 512], f8, tag="vv")
            mt_sb = singles.tile([16, 2, 128], bf16, tag="mt")

            def bc_ap(t):
                return ws_sb[:, _BC0 + t : _BC0 + t + 1]

            def mask2_ap(t):
                return ws_sb[:, _MK0 + 16 * t : _MK0 + 16 * (t + 1)]

            xts = {}     # i -> x tile [128, 4, 1024] f8
            exps = {}    # i -> [128, 2, 1024] f32r
            rds = {}     # i -> [rd_t0, rd_t1] [128, 1] f32
            lcss = {}    # i -> [128, 2, 16] f32r
            rss = {}     # i -> [16, 1024] bf16
            attnfs = {}  # (i, c) -> [128, 2, 512] f8
            pos = {}     # (i, t, c) -> [128, 512] bf16

            def load_x(i, eng=None, split=False):
                t = xtp.tile([128, 4, 1024], f8, tag="xt")
                src = xt[i].rearrange("(k p) n -> p k n", p=128)
                if split:
                    # halves on two queues in parallel (ACT ring idle early)
                    nc.sync.dma_start(out=t[:, 0:2, :], in_=src[:, 0:2, :])
                    nc.scalar.dma_start(out=t[:, 2:4, :], in_=src[:, 2:4, :])
                else:
                    (eng or nc.sync).dma_start(out=t, in_=src)
                xts[i] = t

            def g1_exp_t(i, t):
                """GEMM1 (DoubleRow, k-pairs) + fused exp for one head-half."""
                pa = ps_pap.tile([128, 1024], f32, tag="pa", name="pa")
                for c in range(2):
                    for kp in range(2):
                        nc.tensor.matmul(
                            pa[:, 512 * c : 512 * (c + 1)],
                            lhsT=wc_sb[:, t, 2 * kp : 2 * kp + 2, :],
                            rhs=xts[i][:, 2 * kp : 2 * kp + 2,
                                       512 * c : 512 * (c + 1)],
                            start=(kp == 0),
                            stop=(kp == 1),
                            perf_mode=DR,
                        )
                if i not in exps:
                    exps[i] = expp.tile([128, 2, 1024], f32r, tag="exp",
                                        name="exp")
                    rds[i] = [None, None]
                Dp = smallp.tile([128, 1], f32, tag="Dp", name="Dp")
                nc.scalar.activation(
                    out=exps[i][:, t, :], in_=pa, func=Exp, bias=bc_ap(t),
                    scale=1.0 / 64, accum_out=Dp,
                )
                rd = smallp.tile([128, 1], f32, tag="rd", name="rd")
                with tc.high_priority():
                    nc.vector.reciprocal(rd, Dp)
                rds[i][t] = rd
                if i not in lcss:
                    lcss[i] = smallp.tile([128, 2, 16], f32r, tag="lcs",
                                          name="lcs")
                with tc.high_priority():
                    nc.vector.tensor_scalar_mul(lcss[i][:, t, :], mask2_ap(t), rd)

            def colsum_c(i, c):
                ps_s = ps_sp.tile([16, 512], f32, tag="s")
                for t in range(2):
                    nc.tensor.matmul(
                        ps_s,
                        lhsT=lcss[i][:, t, :],
                        rhs=exps[i][:, t, 512 * c : 512 * (c + 1)],
                        start=(t == 0),
                        stop=(t == 1),
                    )
                if i not in rss:
                    rss[i] = rsp.tile([16, 1024], bf16, tag="rs", name="rs")
                with tc.high_priority():
                    nc.vector.reciprocal(rss[i][:, 512 * c : 512 * (c + 1)], ps_s)

            def outer_tc(i, t, c, pool):
                """po via PE outer matmul into spare psum (fill/drain path:
                skips the bcast-DMA init latency)."""
                po = pool.tile([128, 512], f32, tag="y" if pool is ps_yp
                               else "pa", name="po_ps")
                nc.tensor.matmul(
                    po,
                    lhsT=mt_sb[:, t, :],
                    rhs=rss[i][:, 512 * c : 512 * (c + 1)],
                    start=True,
                    stop=True,
                )
                pos[(i, t, c)] = po

            def outer_pa_t(i, t):
                """Drain path: po for both c in one 2-bank pa tile."""
                po = ps_pap.tile([128, 1024], f32, tag="pa", name="po_pa")
                for c in range(2):
                    nc.tensor.matmul(
                        po[:, 512 * c : 512 * (c + 1)],
                        lhsT=mt_sb[:, t, :],
                        rhs=rss[i][:, 512 * c : 512 * (c + 1)],
                        start=True,
                        stop=True,
                    )
                    pos[(i, t, c)] = po[:, 512 * c : 512 * (c + 1)]

            def bcast_tc(i, t, c, eng):
                """po[p, n] = rs[8t + p//16, 512c+n] via stride-0 DMA."""
                po = pop.tile([128, 512], bf16, tag="po", name="po")
                src = rss[i][8 * t : 8 * t + 8, 512 * c : 512 * (c + 1)] \
                    .unsqueeze(1).broadcast_to([8, 16, 512])
                eng.dma_start(out=po, in_=src)
                pos[(i, t, c)] = po

            def attnf_tc(i, t, c):
                if i not in attnfs:
                    attnfs[i] = attnfp.tile([128, 2, 1024], f8,
                                            tag="attnf", name="attnf")
                with tc.high_priority():
                    nc.vector.scalar_tensor_tensor(
                        out=attnfs[i][:, t, 512 * c : 512 * (c + 1)],
                        in0=exps[i][:, t, 512 * c : 512 * (c + 1)],
                        scalar=rds[i][t],
                        in1=pos[(i, t, c)],
                        op0=mult,
                        op1=mult,
                    )

            def g2_tile(i, j):
                """One n-tile of GEMM2: DoubleRow folds both head-halves."""
                ps_out = ps_yp.tile([128, 512], f32, tag="y")
                nc.tensor.matmul(
                    ps_out,
                    lhsT=attnfs[i][:, :, 128 * j : 128 * (j + 1)],
                    rhs=vv_sb,
                    start=True,
                    stop=True,
                    perf_mode=DR,
                )
                return ps_out

            def evict_tile(i, j, ps_out, yg):
                if evict_split:
                    # column-split every eviction: ACT takes the head, DVE the
                    # tail - balances the two engines and keeps DVE's pieces
                    # too small to block its chain-critical attnf work
                    s = evict_split
                    nc.scalar.activation(out=yg[:, 0 : 512 - s],
                                         in_=ps_out[:, 0 : 512 - s],
                                         func=Copy, bias=0.0, scale=1.0)
                    nc.vector.tensor_copy(yg[:, 512 - s : 512],
                                          ps_out[:, 512 - s : 512])
                    return
                acts = evict_acts if i < NB - 2 else late_evict_acts
                if j in acts:
                    nc.scalar.activation(out=yg, in_=ps_out, func=Copy,
                                         bias=0.0, scale=1.0)
                else:
                    nc.vector.tensor_copy(yg, ps_out)

            def ydma_group(i, g, yg, eng):
                dst = y[i, 256 * g : 256 * (g + 1), :] \
                    .rearrange("(j p) e -> p j e", p=128)
                eng.dma_start(out=dst, in_=yg)

            engs = {"gpsimd": nc.gpsimd, "sync": nc.sync, "scalar": nc.scalar}

            def g2_cleanup(i):
                del exps[i], rds[i], lcss[i], attnfs[i]
                for t in range(2):
                    for c in range(2):
                        del pos[(i, t, c)]

            def g2_steps(i):
                """Yields after each group of n-tiles (one y DMA group)."""
                ng = 8 // ygroup
                for g in range(ng):
                    yg = ygp.tile([128, ygroup, 512], bf16, tag="yg",
                                  name="yg")
                    for jj in range(ygroup):
                        j = ygroup * g + jj
                        ps_out = g2_tile(i, j)
                        evict_tile(i, j, ps_out, yg[:, jj, :])
                    dst = y[i, 128 * ygroup * g : 128 * ygroup * (g + 1), :] \
                        .rearrange("(j p) e -> p j e", p=128)
                    engs[ydma_split[g % len(ydma_split)]].dma_start(
                        out=dst, in_=yg)
                    yield
                g2_cleanup(i)

            def g2_half_drain(i, half):
                """Drain-mode GEMM2: single-tile y DMAs spread over 3 queues,
                evicts alternating ACT/DVE so both engines drain in parallel."""
                ddma = list(drain_dma)
                for jj in range(4):
                    j = 4 * half + jj
                    ps_out = g2_tile(i, j)
                    yg = ygp.tile([128, 512], bf16, tag="ygd", name="ygd")
                    if jj in drain_acts:
                        nc.scalar.activation(out=yg, in_=ps_out, func=Copy,
                                             bias=0.0, scale=1.0)
                    else:
                        nc.vector.tensor_copy(yg, ps_out)
                    dst = y[i, 128 * j : 128 * (j + 1), :] \
                        .rearrange("(o p) e -> p o e", p=128)
                    engs[ddma[jj]].dma_start(out=dst, in_=yg)
                if half == 1:
                    g2_cleanup(i)

            def drain(gen):
                if gen is not None:
                    for _ in gen:
                        pass

            # startup loads: Pool carries wc+ws, SP carries x(0) quarters,
            # ACT (idle until the first exp) carries x(1) and vv
            nc.gpsimd.dma_start(out=wc_sb, in_=wc)
            nc.gpsimd.dma_start(out=ws_sb, in_=ws)
            load_x(0, split=x0_split)
            load_x(1, eng=nc.gpsimd)
            nc.gpsimd.dma_start(out=mt_sb, in_=mt)
            nc.gpsimd.dma_start(out=vv_sb, in_=vv)

            b_engs = [engs[e] for e in bcast_eng]

            # Software pipeline, skew 2 for GEMM2:
            #   iter i: norm(i-1) | G1+exp(i) | G2+evict+ydma(i-2)
            # PE order: colsum(i-1) c0,c1 -> G1(i) t0,t1 -> G2(i-2); this keeps
            # the DVE queue (rs before recipD) and ACT queue (exp before
            # evicts) aligned with data readiness.  The last batch's GEMM2 is
            # folded into its norm iteration (drain shrink): each c-half runs
            # right after its attnf pair, with single-tile y DMAs on 3 queues.
            # PE order inside an iteration: G1(i, t0) first (its pa bank frees
            # as soon as exp(i-1, t0) ran, and it feeds ACT asap), then the
            # norm of i-1, then G1(i, t1), then G2(i-2).  This keeps ACT
            # saturated from the fill onward instead of HOL-blocking batch i
            # behind batch i-1's norm chain.
            for i in range(NB + 1):
                last = (i == NB)
                g2 = g2_steps(i - 2) if 2 <= i else None
                if i < NB:
                    g1_exp_t(i, 0)
                if 1 <= i:
                    j = i - 1
                    if j == 0:
                        # fill: y banks are free, po via PE outer (no DMA wait)
                        colsum_c(j, 0)
                        outer_tc(j, 0, 0, ps_yp)
                        outer_tc(j, 1, 0, ps_yp)
                        attnf_tc(j, 0, 0)
                        attnf_tc(j, 1, 0)
                        colsum_c(j, 1)
                        outer_tc(j, 0, 1, ps_yp)
                        outer_tc(j, 1, 1, ps_yp)
                    elif j == NB - 1:
                        # drain: pa banks are free, po via PE outer
                        colsum_c(j, 0)
                        colsum_c(j, 1)
                        outer_pa_t(j, 0)
                        outer_pa_t(j, 1)
                        attnf_tc(j, 0, 0)
                        attnf_tc(j, 1, 0)
                    elif all_outer:
                        colsum_c(j, 0)
                        outer_tc(j, 0, 0, ps_yp)
                        outer_tc(j, 1, 0, ps_yp)
                        attnf_tc(j, 0, 0)
                        attnf_tc(j, 1, 0)
                        colsum_c(j, 1)
                        outer_tc(j, 0, 1, ps_yp)
                        outer_tc(j, 1, 1, ps_yp)
                    else:
                        colsum_c(j, 0)
                        bcast_tc(j, 0, 0, b_engs[0])
                        bcast_tc(j, 1, 0, b_engs[1])
                        attnf_tc(j, 0, 0)
                        attnf_tc(j, 1, 0)
                        colsum_c(j, 1)
                        bcast_tc(j, 0, 1, b_engs[0])
                        bcast_tc(j, 1, 1, b_engs[1])
                if i < NB:
                    g1_exp_t(i, 1)
                if g2 is not None:
                    next(g2, None)  # 2 G2 tiles + y DMA group 0
                if 1 <= i:
                    attnf_tc(i - 1, 0, 1)
                    attnf_tc(i - 1, 1, 1)
                drain(g2)
                if last:
                    g2_half_drain(NB - 1, 0)
                    g2_half_drain(NB - 1, 1)
                if i + 2 < NB:
                    load_x(i + 2)
    nc.compile()
    _nc_cache[key] = nc
    return nc


def _fold_weights(W_in, b_in, W_mk, b_mk, W_mv, b_mv, W_out, b_out):
    f64 = np.float64
    W_in_r = W_in.astype(f64).reshape(E, H, HD)          # [e, h, d]
    W_out_r = W_out.astype(f64).reshape(H, HD, E)        # [h, d, e]
    Wmk = W_mk.astype(f64)                               # [d, m]
    Wmv = W_mv.astype(f64)                               # [m, d]

    comb = np.einsum("ehd,dm->ehm", W_in_r, Wmk)         # [e, h, m]
    Wcg = comb.reshape(E, 2, 8 * M)                      # [e, t, c]
    # wc_host[p, t, k, c] = Wcg[128k + p, t, c]  (lhsT tile for (t, k))
    wc_host = np.ascontiguousarray(
        Wcg.reshape(4, 128, 2, 128).transpose(1, 2, 0, 3)
    ).astype(np.float32)

    bcomb = np.einsum("hd,dm->hm", b_in.astype(f64).reshape(H, HD), Wmk) + b_mk.astype(f64)
    bc_host = np.ascontiguousarray(bcomb.reshape(2, 128).T).astype(np.float32)  # [p, t]

    Vfull = np.einsum("md,hde->hme", Wmv, W_out_r)       # [h, m, e]
    vv_host = np.ascontiguousarray(
        Vfull.reshape(2, 128, E).transpose(1, 0, 2)
    ).astype(np.float32)                                 # [p, t, e]

    by_host = (
        b_out.astype(f64) + np.einsum("d,hde->e", b_mv.astype(f64), W_out_r)
    ).reshape(1, E).astype(np.float32)

    p = np.arange(128)
    g = np.arange(16)
    mask_host = np.zeros((128, 2, 16), np.float32)
    for t in range(2):
        mask_host[p, t, :] = (g[None, :] == (8 * t + p[:, None] // 16)).astype(np.float32)
    maskT_host = np.ascontiguousarray(mask_host.transpose(2, 1, 0))  # [g, t, p]

    ones_host = np.ones((1, 128), np.float32)
    return wc_host, bc_host, vv_host, by_host, mask_host, maskT_host, ones_host


def _pack_small(bc_h, mask_h):
    ws = np.zeros((128, _WS_COLS), np.float32)
    ws[:, _BC0 : _BC0 + 2] = bc_h
    for t in range(2):
        # mask2 = mask/16: makes rs = 16/s so attnf = 16*attn fits fp8 range
        ws[:, _MK0 + 16 * t : _MK0 + 16 * (t + 1)] = mask_h[:, t, :] / 16.0
    return ws


def build_in_maps(x, W_in, b_in, W_mk, b_mk, W_mv, b_mv, W_out, b_out):
    wc_h, bc_h, vv_h, by_h, mask_h, maskT_h, ones_h = _fold_weights(
        W_in, b_in, W_mk, b_mk, W_mv, b_mv, W_out, b_out
    )
    import ml_dtypes

    f8 = ml_dtypes.float8_e4m3
    # x [B, N, E] -> x^T per batch [B, E, N], fp8 e4m3
    xt_all = np.ascontiguousarray(
        np.asarray(x, dtype=np.float32).transpose(0, 2, 1)
    ).astype(f8)
    wc_q = (wc_h * 64.0).astype(f8)
    vv_q = (vv_h * 64.0).astype(f8)
    ws_h = _pack_small(bc_h, mask_h)
    mt_h = np.ascontiguousarray(maskT_h).astype(ml_dtypes.bfloat16)  # [g, t, p]

    in_maps = []
    for c in range(NCORES):
        in_maps.append(
            {
                "xt": xt_all[BPC * c : BPC * (c + 1)],
                "wc": wc_q,
                "vv": vv_q,
                "ws": ws_h,
                "mt": mt_h,
            }
        )
    return in_maps, by_h


def kernel(x, W_in, b_in, W_mk, b_mk, W_mv, b_mv, W_out, b_out):
    from concourse.bass_utils import run_bass_kernel_spmd

    x, W_in, b_in, W_mk, b_mk, W_mv, b_mv, W_out, b_out = (
        np.asarray(a)
        for a in (x, W_in, b_in, W_mk, b_mk, W_mv, b_mv, W_out, b_out)
    )
    in_maps, by_h = build_in_maps(
        x, W_in, b_in, W_mk, b_mk, W_mv, b_mv, W_out, b_out
    )
    nc = _build_program()

    res = run_bass_kernel_spmd(nc, in_maps, list(range(NCORES)))
    global _last_results
    _last_results = res
    out = np.concatenate(
        [res.results[c]["y"].astype(np.float32) for c in range(NCORES)], axis=0
    )
    out = out / Y_SCALE + by_h  # undo fp8 scales, add folded bias
    return out


_last_results = None

